# revision 1
# baseline (speedup 1.0000x reference)
"""GATv2 (2 layers) + label-GCN kernel for Trainium2, 8-core SPMD.

Sharding: nodes partitioned across 8 cores (6250 each); edges assigned by
destination node. Per-edge gather of source features via indirect DMA from a
replicated (allgathered) xl table; edge-softmax accumulated via dma_scatter_add
of augmented rows [weighted feats (128) | exp-score per head (4) | pad (60)].
Weights + the tiny 52-node label GCN are replicated on every core.
"""
import sys

sys.path.insert(0, "/opt/trn_rl_repo")

import numpy as np
from concourse import bacc, bass, mybir
import concourse.tile as tile
from concourse.bass import IndirectOffsetOnAxis
from concourse.bass_utils import run_bass_kernel_spmd

f32 = mybir.dt.float32
i32 = mybir.dt.int32
i16 = mybir.dt.int16
AF = mybir.ActivationFunctionType
ALU = mybir.AluOpType

N_CORES = 8
N = 50000
NLOC = N // N_CORES          # 6250
NTILE = 49                   # ceil(6250/128)
NPAD = NTILE * 128           # 6272
NT = NPAD * N_CORES          # 50176 rows in allgathered tables
F = 128                      # feature dim = HC
HEADS = 4
EMB = 32
NL = 52                      # label count
AUG = 192                    # augmented scatter row: 128 feats + 4 z + 60 pad
CH = 2048                    # edges per chunk
SLOPE = 0.2

_CACHE = {}


def _build(EP):
    nc = bacc.Bacc(None, target_bir_lowering=False, debug=False,
                   num_devices=N_CORES)

    def din(name, shape, dt=f32):
        return nc.declare_dram_parameter(name, list(shape), dt, isOutput=False)

    # ---- external inputs (per-core) ----
    xT = din("xT", [128, NLOC])                  # local x, transposed
    srcm = din("srcm", [128, EP // 128], i32)    # xl-table row per edge
    dstm = din("dstm", [128, EP // 128], i32)    # xr-table row per edge
    dstw = din("dstw", [128, EP // 16], i16)     # scatter idx, 16-wrapped
    # replicated weights
    Wl0 = din("Wl0", [128, 128]); bl0 = din("bl0", [128, 1])
    Wr0 = din("Wr0", [128, 128]); br0 = din("br0", [128, 1])
    Wl1 = din("Wl1", [128, 128]); bl1 = din("bl1", [128, 1])
    Wr1 = din("Wr1", [128, 128]); br1 = din("br1", [128, 1])
    att0 = din("att0", [128, 128])               # row-replicated att flat
    att1 = din("att1", [128, 128])
    b0r = din("b0r", [128, 128])                 # row-replicated bias
    b1r = din("b1r", [128, 128])
    ident = din("ident", [128, 128])
    # label GCN inputs
    Ssrc = din("Ssrc", [128, 4 * NL]); Sdst = din("Sdst", [128, 4 * NL])
    ewl = din("ewl", [128, 4])
    Wg0 = din("Wg0", [NL, 64]); bg0 = din("bg0", [NL, 64])
    Wg1 = din("Wg1", [64, NL]); bg1 = din("bg1", [NL, NL])
    WfT = din("WfT", [NL, 128]); bfc = din("bfc", [NL, 1])
    lx = din("lx", [1, NL])

    out = nc.declare_dram_parameter("out", [NL, NPAD], f32, isOutput=True)

    # ---- internal DRAM ----
    xl_sh1 = nc.dram_tensor("xl_sh1", [NPAD, F], f32)
    xl_tb1 = nc.dram_tensor("xl_tb1", [NT, F], f32)
    xr_tb1 = nc.dram_tensor("xr_tb1", [NPAD, F], f32)
    aug1 = nc.dram_tensor("aug1", [NPAD, AUG], f32)
    xl_sh2 = nc.dram_tensor("xl_sh2", [NPAD, F], f32)
    xl_tb2 = nc.dram_tensor("xl_tb2", [NT, F], f32)
    xr_tb2 = nc.dram_tensor("xr_tb2", [NPAD, F], f32)
    aug2 = nc.dram_tensor("aug2", [NPAD, AUG], f32)

    with tile.TileContext(nc) as tc:
        with (
            tc.tile_pool(name="const", bufs=1) as cp,
            tc.tile_pool(name="edge", bufs=2) as ep,
            tc.tile_pool(name="fin", bufs=1) as fp,
            tc.tile_pool(name="dense", bufs=2) as dp,
            tc.tile_pool(name="psA", bufs=2, space="PSUM") as psA,
            tc.tile_pool(name="psB", bufs=2, space="PSUM") as psB,
        ):
            # ---------- stage consts into SBUF ----------
            def ld(dram_t, shape, dt=f32, name=None):
                t = cp.tile(list(shape), dt, tag=name)
                nc.sync.dma_start(out=t[:], in_=dram_t[:])
                return t

            xT_sb = ld(xT, [128, NLOC], name="xT_sb")
            srcm_sb = ld(srcm, [128, EP // 128], i32, "srcm_sb")
            dstm_sb = ld(dstm, [128, EP // 128], i32, "dstm_sb")
            dstw_sb = ld(dstw, [128, EP // 16], i16, "dstw_sb")
            Wl0_sb = ld(Wl0, [128, 128], name="Wl0_sb")
            Wr0_sb = ld(Wr0, [128, 128], name="Wr0_sb")
            Wl1_sb = ld(Wl1, [128, 128], name="Wl1_sb")
            Wr1_sb = ld(Wr1, [128, 128], name="Wr1_sb")
            bl0_sb = ld(bl0, [128, 1], name="bl0_sb")
            br0_sb = ld(br0, [128, 1], name="br0_sb")
            bl1_sb = ld(bl1, [128, 1], name="bl1_sb")
            br1_sb = ld(br1, [128, 1], name="br1_sb")
            att0_sb = ld(att0, [128, 128], name="att0_sb")
            att1_sb = ld(att1, [128, 128], name="att1_sb")
            b0r_sb = ld(b0r, [128, 128], name="b0r_sb")
            b1r_sb = ld(b1r, [128, 128], name="b1r_sb")
            id_sb = ld(ident, [128, 128], name="id_sb")
            Ssrc_sb = ld(Ssrc, [128, 4 * NL], name="Ssrc_sb")
            Sdst_sb = ld(Sdst, [128, 4 * NL], name="Sdst_sb")
            ewl_sb = ld(ewl, [128, 4], name="ewl_sb")
            Wg0_sb = ld(Wg0, [NL, 64], name="Wg0_sb")
            bg0_sb = ld(bg0, [NL, 64], name="bg0_sb")
            Wg1_sb = ld(Wg1, [64, NL], name="Wg1_sb")
            bg1_sb = ld(bg1, [NL, NL], name="bg1_sb")
            WfT_sb = ld(WfT, [NL, 128], name="WfT_sb")
            bfc_sb = ld(bfc, [NL, 1], name="bfc_sb")
            lx_sb = ld(lx, [1, NL], name="lx_sb")

            zero_sb = cp.tile([128, AUG], f32, tag="zero_sb")
            nc.vector.memset(zero_sb[:], 0.0)

            # zero the aug accumulators and table pad rows
            for aug_t in (aug1, aug2):
                for j in range(NTILE):
                    nc.sync.dma_start(
                        out=aug_t[j * 128:(j + 1) * 128, :],
                        in_=zero_sb[:],
                    )
            for t in (xl_sh1, xr_tb1, xl_sh2, xr_tb2):
                nc.sync.dma_start(
                    out=t[NLOC:NPAD, :],
                    in_=zero_sb[0:NPAD - NLOC, 0:F],
                )

            # helpers --------------------------------------------------
            def pe_T(in_ap, m=128, n=128, tag="tr"):
                """transpose in_ap [m, n] -> sbuf [n, m]"""
                ps = psB.tile([128, 128], f32, tag="psT")
                nc.tensor.transpose(out=ps[0:n, 0:m], in_=in_ap,
                                    identity=id_sb[0:m, 0:m])
                sb = dp.tile([128, 128], f32, tag=tag)
                nc.vector.tensor_copy(out=sb[0:n, 0:m], in_=ps[0:n, 0:m])
                return sb

            def elu(x_ap, tmp_tag):
                """in-place ELU on x_ap; needs a scratch tile"""
                u = fp.tile([128, NTILE, 128], f32, tag=tmp_tag)
                sh = x_ap.shape
                ua = (u[0:sh[0], 0:sh[1], 0:sh[2]] if len(sh) == 3
                      else u[0:sh[0], 0, 0:sh[1]])
                nc.vector.tensor_scalar_min(out=ua, in0=x_ap, scalar1=0.0)
                nc.scalar.activation(out=ua, in_=ua, func=AF.Exp)
                nc.vector.tensor_scalar_add(out=ua, in0=ua, scalar1=-1.0)
                nc.vector.scalar_tensor_tensor(
                    out=x_ap, in0=x_ap, scalar=0.0, in1=ua,
                    op0=ALU.max, op1=ALU.add)

            # ---------- label GCN (replicated) ----------
            apsum = psA.tile([NL, NL], f32, tag="lab_ps")
            rhsw = cp.tile([128, 4, NL], f32, tag="rhsw")
            for c in range(4):
                nc.vector.tensor_scalar_mul(
                    out=rhsw[:, c, :],
                    in0=Ssrc_sb[:].rearrange("p (c l) -> p c l", c=4)[:, c, :],
                    scalar1=ewl_sb[:, c:c + 1])
                nc.tensor.matmul(
                    out=apsum[:],
                    lhsT=Sdst_sb[:].rearrange("p (c l) -> p c l", c=4)[:, c, :],
                    rhs=rhsw[:, c, :],
                    start=(c == 0), stop=(c == 3))
            A_sb = cp.tile([NL, NL], f32, tag="A_sb")
            nc.vector.tensor_copy(out=A_sb[:], in_=apsum[:])
            deg = cp.tile([NL, 1], f32, tag="deg")
            nc.vector.tensor_reduce(out=deg[:], in_=A_sb[:],
                                    axis=mybir.AxisListType.X, op=ALU.add)
            nc.scalar.activation(out=deg[:], in_=deg[:], func=AF.Sqrt)
            dinv = cp.tile([NL, 1], f32, tag="dinv")
            nc.vector.reciprocal(out=dinv[:], in_=deg[:])
            # row of dinv replicated down partitions
            dT_ps = psA.tile([NL, NL], f32, tag="lab_ps")
            nc.tensor.transpose(out=dT_ps[:], in_=dinv[:].to_broadcast([NL, NL]),
                                identity=id_sb[0:NL, 0:NL])
            dT = cp.tile([NL, NL], f32, tag="dT")
            nc.vector.tensor_copy(out=dT[:], in_=dT_ps[:])
            Nh = cp.tile([NL, NL], f32, tag="Nh")
            nc.vector.scalar_tensor_tensor(
                out=Nh[:], in0=A_sb[:], scalar=dinv[:, 0:1], in1=dT[:],
                op0=ALU.mult, op1=ALU.mult)
            NhT_ps = psA.tile([NL, NL], f32, tag="lab_ps")
            nc.tensor.transpose(out=NhT_ps[:], in_=Nh[:], identity=id_sb[0:NL, 0:NL])
            NhT = cp.tile([NL, NL], f32, tag="NhT")
            nc.vector.tensor_copy(out=NhT[:], in_=NhT_ps[:])
            # L0 = diag(label_x / sum(label_x))
            ssum = cp.tile([1, 1], f32, tag="ssum")
            nc.vector.tensor_reduce(out=ssum[:], in_=lx_sb[:],
                                    axis=mybir.AxisListType.X, op=ALU.add)
            nc.vector.reciprocal(out=ssum[:], in_=ssum[:])
            lxs = cp.tile([1, NL], f32, tag="lxs")
            nc.vector.tensor_scalar_mul(out=lxs[:], in0=lx_sb[:],
                                        scalar1=ssum[:, 0:1])
            lcol_ps = psA.tile([NL, 1], f32, tag="lab_ps")
            nc.tensor.transpose(out=lcol_ps[:], in_=lxs[:],
                                identity=id_sb[0:1, 0:1])
            lcol = cp.tile([NL, 1], f32, tag="lcol")
            nc.vector.tensor_copy(out=lcol[:], in_=lcol_ps[:])
            L0 = cp.tile([NL, NL], f32, tag="L0")
            nc.vector.tensor_scalar_mul(out=L0[:], in0=id_sb[0:NL, 0:NL],
                                        scalar1=lcol[:, 0:1])
            # X0 = L0 @ Wg0 (L0 symmetric)
            x0_ps = psA.tile([NL, 64], f32, tag="lab_ps")
            nc.tensor.matmul(out=x0_ps[:], lhsT=L0[:], rhs=Wg0_sb[:],
                             start=True, stop=True)
            X0 = cp.tile([NL, 64], f32, tag="X0")
            nc.vector.tensor_copy(out=X0[:], in_=x0_ps[:])
            l1_ps = psA.tile([NL, 64], f32, tag="lab_ps")
            nc.tensor.matmul(out=l1_ps[:], lhsT=NhT[:], rhs=X0[:],
                             start=True, stop=True)
            L1 = cp.tile([NL, 64], f32, tag="L1")
            nc.vector.tensor_tensor(out=L1[:], in0=l1_ps[:], in1=bg0_sb[:],
                                    op=ALU.add)
            elu(L1[:], "lab_tmp")
            L1T = pe_T(L1[:], m=NL, n=64, tag="L1T")
            h_ps = psA.tile([NL, NL], f32, tag="lab_ps")
            nc.tensor.matmul(out=h_ps[:], lhsT=L1T[0:64, 0:NL], rhs=Wg1_sb[:],
                             start=True, stop=True)
            Hm = cp.tile([NL, NL], f32, tag="Hm")
            nc.vector.tensor_copy(out=Hm[:], in_=h_ps[:])
            l2_ps = psA.tile([NL, NL], f32, tag="lab_ps")
            nc.tensor.matmul(out=l2_ps[:], lhsT=NhT[:], rhs=Hm[:],
                             start=True, stop=True)
            L2 = cp.tile([NL, NL], f32, tag="L2")
            nc.vector.tensor_tensor(out=L2[:], in0=l2_ps[:], in1=bg1_sb[:],
                                    op=ALU.add)
            elu(L2[:], "lab_tmp")
            wfl_ps = psA.tile([128, NL], f32, tag="lab_ps")
            nc.tensor.matmul(out=wfl_ps[:], lhsT=WfT_sb[:], rhs=L2[:],
                             start=True, stop=True)
            WfL = cp.tile([128, NL], f32, tag="WfL")
            nc.vector.tensor_copy(out=WfL[:], in_=wfl_ps[:])
            bfl_ps = psA.tile([1, NL], f32, tag="lab_ps")
            nc.tensor.matmul(out=bfl_ps[:], lhsT=bfc_sb[:], rhs=L2[:],
                             start=True, stop=True)
            bfl_row = cp.tile([1, NL], f32, tag="bfl_row")
            nc.vector.tensor_copy(out=bfl_row[:], in_=bfl_ps[:])
            bflc_ps = psA.tile([NL, 1], f32, tag="lab_ps")
            nc.tensor.transpose(out=bflc_ps[:], in_=bfl_row[:],
                                identity=id_sb[0:1, 0:1])
            bfL = cp.tile([NL, 1], f32, tag="bfL")
            nc.vector.tensor_copy(out=bfL[:], in_=bflc_ps[:])

            # ---------- layer-1 dense: xl1/xr1 shards ----------
            def dense_from_xT(src_ap_fn, Wl_sb, bl_sb, Wr_sb, br_sb,
                              xl_dst, xr_dst, pref):
                for j in range(NTILE):
                    lo = j * 128
                    m = min(128, NLOC - lo)
                    if m <= 0:
                        break
                    rhs = src_ap_fn(j, m)
                    for (W_sb, b_sb, dst) in ((Wl_sb, bl_sb, xl_dst),
                                              (Wr_sb, br_sb, xr_dst)):
                        ps = psA.tile([128, 128], f32, tag="mm_ps")
                        nc.tensor.matmul(out=ps[:, 0:m], lhsT=W_sb[:], rhs=rhs,
                                         start=True, stop=True)
                        tsb = dp.tile([128, 128], f32, tag=pref + "_t")
                        nc.scalar.activation(out=tsb[:, 0:m], in_=ps[:, 0:m],
                                             func=AF.Identity, bias=b_sb[:, 0:1])
                        rsb = pe_T(tsb[:, 0:m], m=128, n=m, tag=pref + "_r")
                        nc.sync.dma_start(out=dst[lo:lo + m, :],
                                          in_=rsb[0:m, :])

            dense_from_xT(lambda j, m: xT_sb[:, j * 128:j * 128 + m],
                          Wl0_sb, bl0_sb, Wr0_sb, br0_sb,
                          xl_sh1, xr_tb1, "d1")

            nc.gpsimd.collective_compute(
                "AllGather", ALU.bypass,
                replica_groups=[list(range(N_CORES))],
                ins=[xl_sh1[:].opt()], outs=[xl_tb1[:].opt()])

            # ---------- edge phase ----------
            def edge_phase(xl_tb, xr_tb, aug_t, att_sb):
                NCH = EP // CH
                JC = CH // 128  # 16
                for c in range(NCH):
                    xs = ep.tile([128, JC, F], f32, tag="xs")
                    nc.gpsimd.indirect_dma_start(
                        out=xs[:], out_offset=None, in_=xl_tb[:],
                        in_offset=IndirectOffsetOnAxis(
                            ap=srcm_sb[:, c * JC:(c + 1) * JC], axis=0))
                    xd = ep.tile([128, JC, F], f32, tag="xd")
                    nc.gpsimd.indirect_dma_start(
                        out=xd[:], out_offset=None, in_=xr_tb[:],
                        in_offset=IndirectOffsetOnAxis(
                            ap=dstm_sb[:, c * JC:(c + 1) * JC], axis=0))
                    # s = xs + xd ; lrelu = max(0.2 s, s) ; * att
                    nc.vector.tensor_tensor(out=xd[:], in0=xs[:], in1=xd[:],
                                            op=ALU.add)
                    nc.vector.scalar_tensor_tensor(
                        out=xd[:], in0=xd[:], scalar=SLOPE, in1=xd[:],
                        op0=ALU.mult, op1=ALU.max)
                    nc.vector.tensor_tensor(
                        out=xd[:], in0=xd[:],
                        in1=att_sb[:].to_broadcast([128, F, JC]).rearrange(
                            "p f j -> p j f"),
                        op=ALU.mult)
                    e4 = ep.tile([128, JC, HEADS], f32, tag="e4")
                    nc.vector.tensor_reduce(
                        out=e4[:],
                        in_=xd[:].rearrange("p j (h c2) -> p j h c2", h=HEADS),
                        axis=mybir.AxisListType.X, op=ALU.add)
                    w = ep.tile([128, JC, AUG], f32, tag="w")
                    nc.vector.memset(w[:, :, F + HEADS:AUG], 0.0)
                    nc.scalar.activation(out=w[:, :, F:F + HEADS], in_=e4[:],
                                         func=AF.Exp)
                    nc.vector.tensor_tensor(
                        out=w[:, :, 0:F].rearrange("p j (h c2) -> p j h c2",
                                                   h=HEADS),
                        in0=xs[:].rearrange("p j (h c2) -> p j h c2", h=HEADS),
                        in1=w[:, :, F:F + HEADS].to_broadcast(
                            [128, JC, HEADS, EMB]),
                        op=ALU.mult)
                    nc.gpsimd.dma_scatter_add(
                        aug_t[:], w[:],
                        dstw_sb[:, c * (CH // 16):(c + 1) * (CH // 16)],
                        CH, CH, AUG)

            edge_phase(xl_tb1, xr_tb1, aug1, att0_sb)

            # ---------- finalize layer (aug -> h) ----------
            def finalize(aug_t, br_sb_rep):
                augsb = fp.tile([128, NTILE, AUG], f32, tag="augsb")
                nc.sync.dma_start(
                    out=augsb[:],
                    in_=aug_t[:].rearrange("(j p) d -> p j d", p=128))
                rz = fp.tile([128, NTILE, HEADS], f32, tag="rz")
                nc.vector.tensor_scalar_max(out=rz[:],
                                            in0=augsb[:, :, F:F + HEADS],
                                            scalar1=1e-16)
                nc.vector.reciprocal(out=rz[:], in_=rz[:])
                h4 = augsb[:, :, 0:F].rearrange("p j (h c2) -> p j h c2",
                                                h=HEADS)
                nc.vector.tensor_tensor(
                    out=h4, in0=h4,
                    in1=rz[:].to_broadcast([128, NTILE, HEADS, EMB]),
                    op=ALU.mult)
                hdj = augsb[:, :, 0:F].rearrange("p j d -> p d j")
                nc.vector.tensor_tensor(
                    out=hdj, in0=hdj,
                    in1=br_sb_rep[:].to_broadcast([128, F, NTILE]),
                    op=ALU.add)
                elu(augsb[:, :, 0:F], "fin_tmp")
                return augsb

            h1 = finalize(aug1, b0r_sb)

            # ---------- layer-2 dense from h1 ----------
            def dense_from_h(haug, Wl_sb, bl_sb, Wr_sb, br_sb, xl_dst, xr_dst,
                             pref, final=False):
                hTs = []
                for j in range(NTILE):
                    hT = pe_T(haug[:, j, 0:F], m=128, n=128, tag=pref + "_hT")
                    for (W_sb, b_sb, dst) in ((Wl_sb, bl_sb, xl_dst),
                                              (Wr_sb, br_sb, xr_dst)):
                        ps = psA.tile([128, 128], f32, tag="mm_ps")
                        nc.tensor.matmul(out=ps[:], lhsT=W_sb[:], rhs=hT[:],
                                         start=True, stop=True)
                        tsb = dp.tile([128, 128], f32, tag=pref + "_t")
                        nc.scalar.activation(out=tsb[:], in_=ps[:],
                                             func=AF.Identity, bias=b_sb[:, 0:1])
                        rsb = pe_T(tsb[:], m=128, n=128, tag=pref + "_r")
                        nc.sync.dma_start(
                            out=dst[j * 128:(j + 1) * 128, :], in_=rsb[:])

            dense_from_h(h1, Wl1_sb, bl1_sb, Wr1_sb, br1_sb, xl_sh2, xr_tb2,
                         "d2")

            nc.gpsimd.collective_compute(
                "AllGather", ALU.bypass,
                replica_groups=[list(range(N_CORES))],
                ins=[xl_sh2[:].opt()], outs=[xl_tb2[:].opt()])

            edge_phase(xl_tb2, xr_tb2, aug2, att1_sb)
            h2 = finalize(aug2, b1r_sb)

            # ---------- final projection + sigmoid ----------
            for j in range(NTILE):
                hT = pe_T(h2[:, j, 0:F], m=128, n=128, tag="fin_hT")
                ps = psA.tile([NL, 128], f32, tag="mm_ps")
                nc.tensor.matmul(out=ps[:], lhsT=WfL[:], rhs=hT[:],
                                 start=True, stop=True)
                osb = dp.tile([NL, 128], f32, tag="fin_o")
                nc.scalar.activation(out=osb[:], in_=ps[:], func=AF.Sigmoid,
                                     bias=bfL[:, 0:1])
                nc.sync.dma_start(out=out[:, j * 128:(j + 1) * 128],
                                  in_=osb[:])

    nc.compile()
    return nc


def _prep(inputs):
    x = np.asarray(inputs["x"], np.float32)
    ei = np.asarray(inputs["edge_index"])
    src = np.concatenate([ei[0], np.arange(N, dtype=ei.dtype)]).astype(np.int64)
    dst = np.concatenate([ei[1], np.arange(N, dtype=ei.dtype)]).astype(np.int64)
    core = dst // NLOC

    counts = np.bincount(core, minlength=N_CORES)
    EP = int(np.ceil(counts.max() / CH) * CH)

    srcrow_all = (src // NLOC) * NPAD + (src % NLOC)

    per_core = []
    for c in range(N_CORES):
        m = core == c
        s = srcrow_all[m]
        d = (dst[m] - c * NLOC)
        n = s.shape[0]
        sp = np.full(EP, 0, np.int32)
        dp_ = np.full(EP, NLOC, np.int32)   # pad -> dummy row 6250
        sp[:n] = s
        dp_[:n] = d
        srcm = np.ascontiguousarray(sp.reshape(EP // 128, 128).T).astype(np.int32)
        dstm = np.ascontiguousarray(dp_.reshape(EP // 128, 128).T).astype(np.int32)
        w16 = dp_.astype(np.int16).reshape(EP // 16, 16).T
        dstw = np.ascontiguousarray(np.tile(w16, (8, 1)))
        per_core.append((srcm, dstm, dstw))

    # label GCN host prep (index-only): one-hots with self loops appended
    lei = np.asarray(inputs["label_edge_index"])
    lew = np.asarray(inputs["label_edge_weights"], np.float32)
    ls = np.concatenate([lei[0], np.arange(NL, dtype=lei.dtype)])
    ld_ = np.concatenate([lei[1], np.arange(NL, dtype=lei.dtype)])
    ew = np.concatenate([lew, np.ones(NL, np.float32)])
    NE = ls.shape[0]          # 468
    NEP = 512
    Ssrc = np.zeros((NEP, NL), np.float32)
    Sdst = np.zeros((NEP, NL), np.float32)
    ewp = np.zeros(NEP, np.float32)
    Ssrc[np.arange(NE), ls] = 1.0
    Sdst[np.arange(NE), ld_] = 1.0
    ewp[:NE] = ew
    # [512, 52] -> [128, 4, 52] with edge k at [k%128, k//128]
    Ssrc_in = np.ascontiguousarray(
        Ssrc.reshape(4, 128, NL).transpose(1, 0, 2).reshape(128, 4 * NL))
    Sdst_in = np.ascontiguousarray(
        Sdst.reshape(4, 128, NL).transpose(1, 0, 2).reshape(128, 4 * NL))
    ewl_in = np.ascontiguousarray(ewp.reshape(4, 128).T)

    rep = lambda v: np.ascontiguousarray(
        np.tile(np.asarray(v, np.float32).reshape(1, -1), (128, 1)))
    col = lambda v: np.ascontiguousarray(
        np.asarray(v, np.float32).reshape(-1, 1))

    shared = dict(
        Wl0=np.ascontiguousarray(inputs["Wl0"], np.float32),
        Wr0=np.ascontiguousarray(inputs["Wr0"], np.float32),
        Wl1=np.ascontiguousarray(inputs["Wl1"], np.float32),
        Wr1=np.ascontiguousarray(inputs["Wr1"], np.float32),
        bl0=col(inputs["bl0"]), br0=col(inputs["br0"]),
        bl1=col(inputs["bl1"]), br1=col(inputs["br1"]),
        att0=rep(np.asarray(inputs["att0"], np.float32).reshape(-1)),
        att1=rep(np.asarray(inputs["att1"], np.float32).reshape(-1)),
        b0r=rep(inputs["b0"]), b1r=rep(inputs["b1"]),
        ident=np.eye(128, dtype=np.float32),
        Ssrc=Ssrc_in, Sdst=Sdst_in, ewl=ewl_in,
        Wg0=np.ascontiguousarray(inputs["Wg0"], np.float32),
        bg0=np.ascontiguousarray(
            np.tile(np.asarray(inputs["bg0"], np.float32).reshape(1, 64),
                    (NL, 1))),
        Wg1=np.ascontiguousarray(inputs["Wg1"], np.float32),
        bg1=np.ascontiguousarray(
            np.tile(np.asarray(inputs["bg1"], np.float32).reshape(1, NL),
                    (NL, 1))),
        WfT=np.ascontiguousarray(np.asarray(inputs["Wf"], np.float32).T),
        bfc=col(inputs["bf"]),
        lx=np.ascontiguousarray(
            np.asarray(inputs["label_x"], np.float32).reshape(1, NL)),
    )

    in_maps = []
    for c in range(N_CORES):
        srcm, dstm, dstw = per_core[c]
        m = dict(shared)
        m["xT"] = np.ascontiguousarray(x[c * NLOC:(c + 1) * NLOC, :].T)
        m["srcm"] = srcm
        m["dstm"] = dstm
        m["dstw"] = dstw
        in_maps.append(m)
    return EP, in_maps


def kernel(**inputs):
    EP, in_maps = _prep(inputs)
    if EP not in _CACHE:
        _CACHE[EP] = _build(EP)
    nc = _CACHE[EP]
    res = run_bass_kernel_spmd(nc, in_maps, core_ids=list(range(N_CORES)))
    out = np.empty((N, NL), np.float32)
    for c in range(N_CORES):
        out[c * NLOC:(c + 1) * NLOC, :] = res.results[c]["out"][:, :NLOC].T
    return out

